# revision 36
# baseline (speedup 1.0000x reference)
"""DecoderBlock on 8 NeuronCores (Trainium2, Bass/Tile).

Sharding: tensor-parallel attention (2 heads per core, all batches, full
sequence -> identical causal loop structure on every core, SPMD-clean),
then row-parallel over token rows for the output projection, LayerNorm2
and MLP.

Token ownership is SCATTERED so the head->token exchange can be issued as
one AllToAll per batch (overlapped under the next batch's attention):
rank r owns rows {b*2048 + r*256 + i : b in 0..4, i in 0..256}, i.e. the
r-th 256-row slice of every batch. With that ownership, batch b's
AllToAll over ao[b] rows (chunk c = rows c*256..(c+1)*256 -> rank c)
delivers exactly rank r's rows for batch b.

Pipeline per core r (heads {2r, 2r+1}):
  P1  LN1 (mean/rstd only - gamma/beta are folded into Wq/Wk/Wv/W1 and
      their biases host-side) on own 1024 rows -> transpose -> hT_my
      [D, 1024] bf16
  P2  AllGather -> hT_all [8192, 1024]  (block q = [D, tokens of rank q])
  P3  per batch: QT/KT/V projections (2 heads), causal attention with
      both heads interleaved: scoresT [128 keys, h0 512q | h1 512q] in
      one PSUM tile (the two score matmuls use PE row groups 0-63 /
      64-127 and run concurrently), one Exp covers both heads,
      probs @ V_aug (ones column) gives unnormalized out + softmax
      denominator; normalize; store batch rows to ao_my[b]; issue
      AllToAll for batch b immediately (overlaps later batches).
  P5  out-proj + residual (x2 kept in SBUF as bf16) + LN2 + MLP +
      residual on own rows -> out_my
"""

import numpy as np
import ml_dtypes

B, S, D, H, HD = 4, 2048, 1024, 16, 64
R = 8                       # cores
M = (B * S) // R            # 1024 token rows per core
SC = M // B                 # 256 rows per (batch, rank)
DM = D * 4                  # MLP hidden 4096
NDT = D // 128              # 8 d-tiles
NMO = DM // 128             # 32 mlp-hidden tiles
NT = M // 128               # 8 row-tiles per core
NSK = S // 128              # 16 key tiles per batch
EPS = 1e-5
BF = ml_dtypes.bfloat16

_CACHE = {}


def _split_multi_waits(nc):
    """This walrus build allows only ONE sync-wait per instruction
    (setupSyncWait: 'Too many sync wait commands'). Move extra waits onto
    same-engine nops inserted immediately before the instruction — the
    engine executes the nop waits first, preserving ordering."""
    import concourse.mybir as mybir

    for bb in nc.main_func.blocks:
        orig = list(bb.instructions)
        if not any(
            i.sync_info is not None and len(i.sync_info.on_wait) > 1
            for i in orig
        ):
            continue
        new_list = []
        for inst in orig:
            si = inst.sync_info
            if si is not None and len(si.on_wait) > 1:
                waits = list(si.on_wait)
                del si.on_wait[:]
                si.on_wait.append(waits[-1])
                for w in waits[:-1]:
                    n = nc.engines[inst.engine].nop(
                        nofuse=True, hint="wsplit"
                    )
                    # the builder appended the nop somewhere; pull it out
                    cb = nc.cur_bb.bb
                    cb.instructions.remove(n.ins)
                    n.ins.sync_info = mybir.SyncInfo(on_wait=[w], on_update=[])
                    new_list.append(n.ins)
            new_list.append(inst)
        del bb.instructions[:]
        for i in new_list:
            bb.instructions.append(i)


def _build_program(mlp_act=None, no_collectives=False, direct_ag=False):
    # direct_ag=True (rank-indexed DMA write into the Shared buffer + tiny
    # AllReduce barrier) was measured FLAKY on hardware: the dynamic-offset
    # write does not reliably land in the shared scratchpad, leaving stale/
    # garbage data for peers (NaN output). Keep the classic AllGather.
    import concourse.bass as bass
    import concourse.mybir as mybir
    import concourse.tile as tile
    from concourse.tile import add_dep_helper

    f32 = mybir.dt.float32
    bf16 = mybir.dt.bfloat16
    Alu = mybir.AluOpType
    Act = mybir.ActivationFunctionType
    if mlp_act is None:
        mlp_act = Act.Gelu

    nc = bass.Bass("TRN2", target_bir_lowering=False, debug=False, num_devices=R)

    # ---- I/O -------------------------------------------------------------
    x_my = nc.dram_tensor("x_my", [M, D], f32, kind="ExternalInput")
    # x_myb = x_my + bo, prepared host-side: P5's residual add needs
    # x + bo and folding bo here keeps it a single tensor_tensor
    x_myb = nc.dram_tensor("x_myb", [M, D], f32, kind="ExternalInput")
    wq = nc.dram_tensor("wq", [128, D], bf16, kind="ExternalInput")
    wk = nc.dram_tensor("wk", [128, D], bf16, kind="ExternalInput")
    wv = nc.dram_tensor("wv", [128, D], bf16, kind="ExternalInput")
    wo = nc.dram_tensor("wo", [128, NDT, D], bf16, kind="ExternalInput")
    w1 = nc.dram_tensor("w1", [NMO, 128, D], bf16, kind="ExternalInput")
    w2 = nc.dram_tensor("w2", [128, NMO, D], bf16, kind="ExternalInput")
    bq2 = nc.dram_tensor("bq2", [128, 1], f32, kind="ExternalInput")
    bk2 = nc.dram_tensor("bk2", [128, 1], f32, kind="ExternalInput")
    bv_bc = nc.dram_tensor("bv_bc", [128, 130], f32, kind="ExternalInput")
    b2_bc = nc.dram_tensor("b2_bc", [128, D], f32, kind="ExternalInput")
    b1_col = nc.dram_tensor("b1_col", [128, NMO], f32, kind="ExternalInput")
    tri = nc.dram_tensor("tri", [128, 128], bf16, kind="ExternalInput")
    iden = nc.dram_tensor("iden", [128, 128], bf16, kind="ExternalInput")
    out_my = nc.dram_tensor("out_my", [M, D], f32, kind="ExternalOutput")

    with tile.TileContext(nc) as tc:
        # ---- DRAM intermediates (collective bounce buffers) --------------
        with tc.tile_pool(name="dram", bufs=1, space="DRAM") as dram:
            hT_my = dram.tile([D, M], bf16, tag="hT_my")
            hT_all = dram.tile([R * D, M], bf16, tag="hT_all",
                               addr_space="Shared")
            bar_in = dram.tile([128, 1], f32, tag="bar_in")
            bar_out = dram.tile([128, 1], f32, tag="bar_out")
            ao_my_b = [dram.tile([S, 128], bf16, tag=f"ao_my_{b}",
                                 name=f"ao_my_{b}") for b in range(B)]
            ao_all_b = [dram.tile([S, 128], bf16, tag=f"ao_all_{b}",
                                  name=f"ao_all_{b}") for b in range(B)]

            with tc.tile_pool(name="consts", bufs=1) as consts:
                iden_sb = consts.tile([128, 128], bf16, tag="iden")
                nc.sync.dma_start(out=iden_sb, in_=iden[:, :])
                tri_sb = consts.tile([128, 128], bf16, tag="tri")
                nc.sync.dma_start(out=tri_sb, in_=tri[:, :])
                bq_sb = consts.tile([128, 1], f32, tag="bq")
                nc.sync.dma_start(out=bq_sb, in_=bq2[:, :])
                bk_sb = consts.tile([128, 1], f32, tag="bk")
                nc.sync.dma_start(out=bk_sb, in_=bk2[:, :])
                bvbc_sb = consts.tile([128, 130], f32, tag="bv")
                nc.sync.dma_start(out=bvbc_sb, in_=bv_bc[:, :])
                b1_sb = consts.tile([128, NMO], f32, tag="b1")
                nc.sync.dma_start(out=b1_sb, in_=b1_col[:, :])
                eps_sb = consts.tile([128, 1], f32, tag="eps")
                nc.vector.memset(eps_sb, EPS)
                wq_sb = consts.tile([128, D], bf16, tag="wq")
                nc.sync.dma_start(out=wq_sb, in_=wq[:, :])
                wk_sb = consts.tile([128, D], bf16, tag="wk")
                nc.sync.dma_start(out=wk_sb, in_=wk[:, :])
                wv_sb = consts.tile([128, D], bf16, tag="wv")
                nc.sync.dma_start(out=wv_sb, in_=wv[:, :])

                def layer_norm_tile(pool, small, xt):
                    """xt: [128, D] -> (x-mu)*rstd as bf16 tile (gamma/beta
                    are folded into the downstream weights host-side)."""
                    stats = small.tile([128, 2, 6], f32, tag="stats")
                    nc.vector.bn_stats(out=stats[:, 0, :], in_=xt[:, 0:512])
                    nc.vector.bn_stats(out=stats[:, 1, :], in_=xt[:, 512:1024])
                    mv = small.tile([128, 2], f32, tag="mv")
                    nc.vector.bn_aggr(out=mv, in_=stats)
                    std = small.tile([128, 1], f32, tag="std")
                    nc.scalar.activation(
                        out=std, in_=mv[:, 1:2], func=Act.Sqrt,
                        bias=eps_sb[:, :], scale=1.0,
                    )
                    rstd = small.tile([128, 1], f32, tag="rstd")
                    nc.vector.reciprocal(out=rstd, in_=std)
                    # (x-mu)*rstd on the Scalar engine: bias is -mu*rstd so
                    # ACT computes x*rstd + (-mu*rstd).
                    nm = small.tile([128, 1], f32, tag="nm")
                    nc.vector.tensor_scalar(
                        out=nm, in0=mv[:, 0:1],
                        scalar1=rstd, scalar2=-1.0,
                        op0=Alu.mult, op1=Alu.mult,
                    )
                    hpre = pool.tile([128, D], bf16, tag="hpre")
                    nc.scalar.activation(
                        out=hpre, in_=xt, func=Act.Identity,
                        bias=nm[:, :], scale=rstd[:, :],
                    )
                    return hpre

                # ========== P1: LN1 on own rows, transpose, write hT_my ====
                with (
                    tc.tile_pool(name="p1", bufs=4) as p1,
                    tc.tile_pool(name="p1s", bufs=6) as p1s,
                    tc.tile_pool(name="p1h", bufs=1) as p1h,
                    tc.tile_pool(name="p1ps", bufs=4, space="PSUM") as p1ps,
                ):
                    hT_sb = p1h.tile([128, NDT, M], bf16, tag="hT")
                    for t in range(NT):
                        xt = p1.tile([128, D], f32, tag="xt")
                        nc.sync.dma_start(
                            out=xt, in_=x_my[t * 128:(t + 1) * 128, :]
                        )
                        hpre = layer_norm_tile(p1, p1s, xt)
                        for dt in range(NDT):
                            pt = p1ps.tile([128, 128], bf16, tag="pt")
                            nc.tensor.transpose(
                                pt, hpre[:, dt * 128:(dt + 1) * 128], iden_sb
                            )
                            # split the PSUM->SBUF copies to balance ACT
                            # (sqrt+apply) against DVE (bn_stats) in P1
                            if dt < 5:
                                nc.scalar.copy(
                                    out=hT_sb[:, dt, t * 128:(t + 1) * 128],
                                    in_=pt,
                                )
                            else:
                                nc.vector.tensor_copy(
                                    out=hT_sb[:, dt, t * 128:(t + 1) * 128],
                                    in_=pt,
                                )
                # ========== P2: publish hT to the shared buffer ==========
                # Either a classic AllGather, or (direct_ag) each rank
                # DMA-writes its own block of the ONE shared hT_all buffer
                # at a partition_id-dependent offset, then a tiny AllReduce
                # acts as the cross-rank barrier before any rank reads.
                ag_gate = None      # instruction every hT_all read must wait on
                if direct_ag and not no_collectives:
                    pid = nc.sync.partition_id()
                    w = nc.sync.dma_start(
                        out=hT_all[bass.ds(pid * D, D), :].rearrange(
                            "(dt p) t -> p dt t", p=128),
                        in_=hT_sb,
                    )
                    bar_sb = consts.tile([128, 1], f32, tag="bar")
                    nc.vector.memset(bar_sb, 1.0)
                    nc.sync.dma_start(out=bar_in[:, :], in_=bar_sb)
                    ag_gate = nc.gpsimd.collective_compute(
                        "AllReduce",
                        mybir.AluOpType.add,
                        replica_groups=[list(range(R))],
                        ins=[bar_in.opt()],
                        outs=[bar_out.opt()],
                    )
                    add_dep_helper(ag_gate.ins, w.ins, sync=True,
                                   reason="barrier waits on hT block write")
                else:
                    nc.sync.dma_start(
                        out=hT_my[:, :].rearrange("(dt p) t -> p dt t", p=128),
                        in_=hT_sb,
                    )
                    if no_collectives:
                        nc.sync.dma_start(out=hT_all[0:D, :], in_=hT_my[:, :])
                    else:
                        nc.gpsimd.collective_compute(
                            "AllGather",
                            mybir.AluOpType.bypass,
                            replica_groups=[list(range(R))],
                            ins=[hT_my.opt()],
                            outs=[hT_all.opt()],
                        )

                # ========== P3: attention, per batch ======================
                with (
                    tc.tile_pool(name="aht", bufs=2) as aht,
                    tc.tile_pool(name="aqk", bufs=2) as aqk,
                    tc.tile_pool(name="aex", bufs=5) as aex,
                    tc.tile_pool(name="asm", bufs=8) as asm,
                    tc.tile_pool(name="aob", bufs=2) as aob,
                    tc.tile_pool(name="psA", bufs=2, space="PSUM") as psA,
                    tc.tile_pool(name="psO", bufs=2, space="PSUM") as psO,
                    tc.tile_pool(name="psQ", bufs=2, space="PSUM") as psQ,
                ):
                    for b in range(B):
                        # batch b keys/queries: token tb = j*256+i lives in
                        # rank j's hT_all block, columns [b*256, (b+1)*256)
                        hT_b = aht.tile([128, NDT, S], bf16, tag="hT_b")
                        for j in range(R):
                            ld = nc.sync.dma_start(
                                out=hT_b[:, :, j * SC:(j + 1) * SC],
                                in_=hT_all[j * D:(j + 1) * D,
                                           b * SC:(b + 1) * SC].rearrange(
                                    "(dt p) t -> p dt t", p=128
                                ),
                            )
                            if ag_gate is not None:
                                add_dep_helper(ld.ins, ag_gate.ins, sync=True,
                                               reason="hT read waits on barrier")
                        # --- QT / KT: [128 (2 heads x 64), S] bf16 --------
                        QT = aqk.tile([128, S], bf16, tag="QT")
                        KT = aqk.tile([128, S], bf16, tag="KT")
                        for dst, wsb, bsb in ((QT, wq_sb, bq_sb), (KT, wk_sb, bk_sb)):
                            for sl in range(S // 512):
                                ps = psQ.tile([128, 512], f32, tag="psQ")
                                for dt in range(NDT):
                                    nc.tensor.matmul(
                                        ps,
                                        lhsT=wsb[:, dt * 128:(dt + 1) * 128],
                                        rhs=hT_b[:, dt, sl * 512:(sl + 1) * 512],
                                        start=(dt == 0), stop=(dt == NDT - 1),
                                    )
                                nc.vector.tensor_scalar_add(
                                    out=dst[:, sl * 512:(sl + 1) * 512],
                                    in0=ps, scalar1=bsb[:, :],
                                )
                        # --- V (+ones column): [128, 16, 130] bf16 --------
                        va = aqk.tile([128, NSK, 130], bf16, tag="va")
                        nc.vector.memset(va, 1.0)
                        for sk in range(NSK):
                            ps = psQ.tile([128, 128], f32, tag="psQ")
                            for dt in range(NDT):
                                nc.tensor.matmul(
                                    ps,
                                    lhsT=hT_b[:, dt, sk * 128:(sk + 1) * 128],
                                    rhs=wv_sb[:, dt * 128:(dt + 1) * 128],
                                    start=(dt == 0), stop=(dt == NDT - 1),
                                )
                            nc.vector.tensor_tensor(
                                out=va[:, sk, :].rearrange(
                                    "p (h c) -> p h c", c=65)[:, :, 0:64],
                                in0=ps.rearrange("p (h c) -> p h c", c=64),
                                in1=bvbc_sb[:, :].rearrange(
                                    "p (h c) -> p h c", c=65)[:, :, 0:64],
                                op=Alu.add,
                            )
                        # --- causal attention, both heads interleaved -----
                        # scoresT tile [128 keys, h0 512q | h1 512q]: the two
                        # score matmuls hit PE row groups 0-63 / 64-127 and
                        # run concurrently; one 1024-wide Exp serves both.
                        aosb = aob.tile([128, NSK, 128], bf16, tag="aosb")
                        for g in range(S // 512):
                            # two sq tiles share one PSUM bank: [sq-even
                            # h0|h1, sq-odd h0|h1] = [128, 260] fits a 2KB
                            # bank; slot i holds sq pair (2i, 2i+1)
                            pso = [
                                psO.tile([128, 260], f32, tag="pso",
                                         name=f"pso_{b}_{g}_{i}")
                                for i in range(2)
                            ]

                            def pso_sl(sq, h2):
                                base = (sq % 2) * 130 + h2 * 65
                                return pso[sq // 2][:, base:base + 65]
                            n_sk = 4 * g + 4
                            for sk in range(n_sk):
                                # queries below the diagonal never read key
                                # tile sk: narrow scores+exp to the live
                                # query range [sq0*128, 512)
                                sq0 = max(0, sk - 4 * g)
                                pss = psA.tile([128, 1024], f32, tag="psA")
                                for h2 in range(2):
                                    hofs = h2 * 64
                                    nc.tensor.matmul(
                                        pss[:, h2 * 512 + sq0 * 128:
                                            (h2 + 1) * 512],
                                        lhsT=KT[hofs:hofs + 64,
                                                sk * 128:(sk + 1) * 128],
                                        rhs=QT[hofs:hofs + 64,
                                               g * 512 + sq0 * 128:
                                               (g + 1) * 512],
                                        start=True, stop=True,
                                    )
                                ex = aex.tile([128, 1024], bf16, tag="ex")
                                if sq0 == 0:
                                    nc.scalar.activation(
                                        out=ex, in_=pss, func=Act.Exp
                                    )
                                else:
                                    nc.scalar.activation(
                                        out=ex[:, :].rearrange(
                                            "p (h w) -> p h w", h=2
                                        )[:, :, sq0 * 128:512],
                                        in_=pss[:, :].rearrange(
                                            "p (h w) -> p h w", h=2
                                        )[:, :, sq0 * 128:512],
                                        func=Act.Exp,
                                    )
                                for sq in range(4):
                                    sqt = 4 * g + sq
                                    if sk > sqt:
                                        continue
                                    for h2 in range(2):
                                        exs = ex[:, h2 * 512 + sq * 128:
                                                 h2 * 512 + (sq + 1) * 128]
                                        if sk == sqt:
                                            nc.vector.tensor_mul(
                                                out=exs, in0=exs,
                                                in1=tri_sb
                                            )
                                        # start=True clears has_written for
                                        # the WHOLE bank, so only the bank's
                                        # program-order-first group (sk==0,
                                        # even sq, h0) may open it; the other
                                        # three groups' first matmuls land as
                                        # overwrite-where-unset.
                                        nc.tensor.matmul(
                                            pso_sl(sq, h2),
                                            lhsT=exs,
                                            rhs=va[:, sk,
                                                   h2 * 65:(h2 + 1) * 65],
                                            start=(sk == 0 and h2 == 0
                                                   and sq % 2 == 0),
                                            stop=(sk == sqt),
                                        )
                            for sq in range(4):
                                sqt = 4 * g + sq
                                for h2 in range(2):
                                    sl65 = pso_sl(sq, h2)
                                    rec = asm.tile([128, 1], f32, tag="rec")
                                    nc.vector.reciprocal(
                                        out=rec, in_=sl65[:, 64:65]
                                    )
                                    nc.vector.tensor_scalar_mul(
                                        out=aosb[:, sqt, h2 * 64:(h2 + 1) * 64],
                                        in0=sl65[:, 0:64],
                                        scalar1=rec,
                                    )
                        nc.sync.dma_start(
                            out=ao_my_b[b][:, :].rearrange(
                                "(st p) c -> p st c", p=128
                            ),
                            in_=aosb,
                        )
                        # ---- per-batch AllToAll (overlaps later batches) -
                        if no_collectives:
                            nc.sync.dma_start(out=ao_all_b[b][:, :],
                                              in_=ao_my_b[b][:, :])
                        else:
                            nc.gpsimd.collective_compute(
                                "AllToAll",
                                mybir.AluOpType.bypass,
                                replica_groups=[list(range(R))],
                                ins=[ao_my_b[b].opt()],
                                outs=[ao_all_b[b].opt()],
                            )

                # ========== P5: out-proj + LN2 + MLP on own rows ==========
                with (
                    tc.tile_pool(name="p5", bufs=2) as p5,
                    tc.tile_pool(name="p5c", bufs=3) as p5c,
                    tc.tile_pool(name="p5s", bufs=4) as p5s,
                    tc.tile_pool(name="p5big", bufs=1) as p5big,
                    tc.tile_pool(name="psB", bufs=2, space="PSUM") as psB,
                    tc.tile_pool(name="psT", bufs=2, space="PSUM") as psT,
                ):
                    h2T = p5big.tile([128, NDT, M], bf16, tag="h2T")
                    x2sb = p5big.tile([128, NT, D], bf16, tag="x2sb")
                    with (
                        tc.tile_pool(name="p5a", bufs=1) as p5a,
                        tc.tile_pool(name="p5a2", bufs=2) as p5a2,
                    ):
                        aoT = p5a.tile([128, NDT, M], bf16, tag="aoT")
                        wo_sb = p5a.tile([128, NDT, D], bf16, tag="wo")
                        nc.sync.dma_start(out=wo_sb, in_=wo[:, :, :])
                        # one DMA per batch as its AllToAll lands, then
                        # transpose per (source rank j, half): aoT free slot
                        # for token (b, i) is b*256 + i.
                        for b in range(B):
                            at_b = p5a2.tile([128, NSK, 128], bf16,
                                             tag="at_b")
                            nc.sync.dma_start(
                                out=at_b,
                                in_=ao_all_b[b][:, :].rearrange(
                                    "(t p) c -> p t c", p=128),
                            )
                            for j in range(R):
                                for t2 in range(2):
                                    pt = psT.tile([128, 128], bf16, tag="pt")
                                    nc.tensor.transpose(
                                        pt, at_b[:, j * 2 + t2, :], iden_sb
                                    )
                                    nc.vector.tensor_copy(
                                        out=aoT[:, j,
                                                b * SC + t2 * 128:
                                                b * SC + (t2 + 1) * 128],
                                        in_=pt,
                                    )
                        for t in range(NT):
                            xt = p5c.tile([128, D], f32, tag="xt5")
                            nc.sync.dma_start(
                                out=xt, in_=x_myb[t * 128:(t + 1) * 128, :]
                            )
                            psp = psB.tile([128, D], f32, tag="psB")
                            for sl in range(2):
                                for dt in range(NDT):
                                    nc.tensor.matmul(
                                        psp[:, sl * 512:(sl + 1) * 512],
                                        lhsT=aoT[:, dt, t * 128:(t + 1) * 128],
                                        rhs=wo_sb[:, dt, sl * 512:(sl + 1) * 512],
                                        start=(dt == 0), stop=(dt == NDT - 1),
                                    )
                            nc.vector.tensor_tensor(
                                out=x2sb[:, t, :], in0=psp, in1=xt, op=Alu.add,
                            )
                            h2pre = layer_norm_tile(p5c, p5s, x2sb[:, t, :])
                            for dt in range(NDT):
                                pt = psT.tile([128, 128], bf16, tag="pt")
                                nc.tensor.transpose(
                                    pt, h2pre[:, dt * 128:(dt + 1) * 128],
                                    iden_sb,
                                )
                                nc.scalar.copy(
                                    out=h2T[:, dt, t * 128:(t + 1) * 128],
                                    in_=pt,
                                )
                    # ---- MLP -------------------------------------------
                    with (
                        tc.tile_pool(name="p5m", bufs=1) as p5m,
                        tc.tile_pool(name="w1p", bufs=3) as w1p,
                    ):
                        m_sb = p5m.tile([128, NMO, M], bf16, tag="m")
                        w2_sb = p5m.tile([128, NMO, D], bf16, tag="w2")
                        nc.sync.dma_start(out=w2_sb, in_=w2[:, :, :])
                        b2bc_sb = p5m.tile([128, D], f32, tag="b2bc")
                        nc.sync.dma_start(out=b2bc_sb, in_=b2_bc[:, :])
                        # sl-outer: the first token half (batches 0/1) runs
                        # all 32 mo tiles without waiting for the batch-3
                        # AllToAll tail that gates the second half of h2T.
                        # Costs a second pass over w1 (DMA has slack).
                        for sl in range(2):
                            for mo in range(NMO):
                                w1t = w1p.tile([128, D], bf16, tag="w1t")
                                nc.sync.dma_start(out=w1t, in_=w1[mo, :, :])
                                psm = psB.tile([128, 512], f32, tag="psM")
                                for dt in range(NDT):
                                    nc.tensor.matmul(
                                        psm,
                                        lhsT=w1t[:, dt * 128:(dt + 1) * 128],
                                        rhs=h2T[:, dt, sl * 512:(sl + 1) * 512],
                                        start=(dt == 0), stop=(dt == NDT - 1),
                                    )
                                nc.scalar.activation(
                                    out=m_sb[:, mo, sl * 512:(sl + 1) * 512],
                                    in_=psm, func=mlp_act,
                                    bias=b1_sb[:, mo:mo + 1], scale=1.0,
                                )
                        for t in range(NT):
                            psy = psB.tile([128, D], f32, tag="psB")
                            for sl in range(2):
                                for mo in range(NMO):
                                    nc.tensor.matmul(
                                        psy[:, sl * 512:(sl + 1) * 512],
                                        lhsT=m_sb[:, mo, t * 128:(t + 1) * 128],
                                        rhs=w2_sb[:, mo, sl * 512:(sl + 1) * 512],
                                        start=(mo == 0), stop=(mo == NMO - 1),
                                    )
                            ot = p5.tile([128, D], f32, tag="ot")
                            nc.vector.tensor_tensor(
                                out=ot, in0=psy, in1=x2sb[:, t, :], op=Alu.add,
                            )
                            # reuse the xt5 slots (dead after out-proj)
                            ot2 = p5c.tile([128, D], f32, tag="xt5")
                            nc.gpsimd.tensor_tensor(
                                out=ot2, in0=ot, in1=b2bc_sb, op=Alu.add,
                            )
                            nc.sync.dma_start(
                                out=out_my[t * 128:(t + 1) * 128, :], in_=ot2
                            )
    _split_multi_waits(nc)
    return nc


def _prep_inputs(x, Wq, Wk, Wv, bq, bk, bv, Wo, bo, W1, b1, W2, b2, gamma, beta):
    """Shard + cast host-side; returns list of per-core input dicts.

    gamma/beta (the shared pre-norm affine) are folded into Wq/Wk/Wv/W1 and
    their biases: h = hhat*gamma + beta with hhat = (x-mu)*rstd, so
    h @ W + b = hhat @ (gamma*W) + (beta @ W + b).

    Token ownership is scattered: rank r owns rows {b*2048 + r*256 + i}.
    """
    gam = np.asarray(gamma, np.float64)
    bet = np.asarray(beta, np.float64)
    xf = np.ascontiguousarray(
        np.asarray(x, np.float32).reshape(B, R, SC, D))
    tri = np.triu(np.ones((128, 128), np.float32)).astype(BF)
    iden = np.eye(128, dtype=np.float32).astype(BF)

    W1g = np.asarray(W1, np.float64) * gam[:, None]          # [D, 4D]
    b1e = (np.asarray(b1, np.float64) + bet @ np.asarray(W1, np.float64))
    b1_col = np.ascontiguousarray(
        b1e.reshape(NMO, 128).T, dtype=np.float32)
    b2_bc = np.ascontiguousarray(
        np.broadcast_to(np.asarray(b2, np.float32), (128, D)))
    xfb = xf + np.asarray(bo, np.float32)      # x + bo for the P5 residual
    wo_t = np.ascontiguousarray(
        np.asarray(Wo, np.float32).reshape(NDT, 128, D).transpose(1, 0, 2)
    ).astype(BF)
    w1_t = np.ascontiguousarray(
        W1g.astype(np.float32).reshape(NDT, 128, NMO, 128).transpose(
            2, 1, 0, 3).reshape(NMO, 128, D)).astype(BF)
    w2_t = np.ascontiguousarray(
        np.asarray(W2, np.float32).reshape(NMO, 128, D).transpose(1, 0, 2)
    ).astype(BF)

    in_maps = []
    for r in range(R):
        h0, h1 = 2 * r, 2 * r + 1
        Wq2 = np.concatenate([Wq[h0], Wq[h1]], axis=1).astype(np.float64)
        Wk2 = np.concatenate([Wk[h0], Wk[h1]], axis=1).astype(np.float64)
        Wv2 = np.concatenate([Wv[h0], Wv[h1]], axis=1).astype(np.float64)

        def arr_t(w2d):  # [D, 128] -> [128, D] with d-tiles along free dim
            return np.ascontiguousarray(
                w2d.astype(np.float32).reshape(NDT, 128, 128).transpose(
                    1, 0, 2).reshape(128, D)
            ).astype(BF)

        bq2 = ((np.concatenate([bq[h0], bq[h1]]) + bet @ Wq2) * 0.125
               ).astype(np.float32).reshape(128, 1)
        bk2 = (np.concatenate([bk[h0], bk[h1]]) + bet @ Wk2
               ).astype(np.float32).reshape(128, 1)
        bve = np.concatenate([bv[h0], bv[h1]]) + bet @ Wv2
        bv_bc = np.zeros((128, 130), np.float32)
        bv_bc[:, 0:64] = bve[0:64].astype(np.float32)
        bv_bc[:, 65:129] = bve[64:128].astype(np.float32)
        in_maps.append({
            "x_my": np.ascontiguousarray(xf[:, r].reshape(M, D)),
            "x_myb": np.ascontiguousarray(xfb[:, r].reshape(M, D)),
            "wq": arr_t(Wq2 * gam[:, None] * 0.125),
            "wk": arr_t(Wk2 * gam[:, None]),
            "wv": arr_t(Wv2 * gam[:, None]),
            "wo": wo_t, "w1": w1_t, "w2": w2_t,
            "bq2": bq2, "bk2": bk2, "bv_bc": bv_bc,
            "b2_bc": b2_bc, "b1_col": b1_col,
            "tri": tri, "iden": iden,
        })
    return in_maps


def kernel(**inputs):
    inputs = {k: np.asarray(v) for k, v in inputs.items()}
    in_maps = _prep_inputs(**inputs)
    if "nc" not in _CACHE:
        _CACHE["nc"] = _build_program()
    from concourse.bass_utils import run_bass_kernel_spmd
    res = run_bass_kernel_spmd(_CACHE["nc"], in_maps, list(range(R)))
    _CACHE["last_res"] = res
    out = np.empty((B, R, SC, D), dtype=np.float32)
    for r in range(R):
        out[:, r] = res.results[r]["out_my"].reshape(B, SC, D)
    return np.ascontiguousarray(out.reshape(B, S, D))
